# revision 17
# baseline (speedup 1.0000x reference)
"""Bahdanau additive-attention pooling for Trainium2 (Bass/Tile).

Reference math (per batch):
    q = x @ Wt + bh; k = x @ Wx                             [L, U]
    e[i,j] = sum_u Wa[u] * tanh(q[i,u] + k[j,u])            (+ ba, dropped --
                                                             softmax shift-inv)
    v = softmax_j(e) @ x                                    [L, D]

Key trick: tanh is replaced by a fitted expansion
    tanh(s) ~= ALIN*s + sum_m CFIT[m] sin(OMEGA[m] s)
which SEPARATES over s = q + k:
    sin(w(q+k)) = sin(wq)cos(wk) + cos(wq)sin(wk)
so e becomes one matmul over F = 2*M*U = 384 trig features:
    e[i,j] ~= sum_f Gq[i,f]*Fk[j,f] + ALIN*(A_i + B_j)
    Gq[i,(m,u,ph)] = c_m*Wa_u*trig_ph(w_m q[i,u]),  Fk = cotrig_ph(w_m k[j,u])
The linear term's A_i part is constant per softmax row (drops out); B_j =
ALIN*(k@Wa)_j rides for free as a per-partition bias on the exp activation.
This removes the L*L*U elementwise tanh volume (the old ScalarE bottleneck,
~110us/core); scores cost one PE matmul with contraction dim 384.

The ScalarE Sin spline is only valid on [-pi, pi] (no hardware range
reduction), so projection tiles are folded with chained DVE add_range_wrap
ops: block0 rows (m=0..3, |w|<=1.83) need one wrap (plus one shifted wrap
for the cos phase); block1 rows (m=4,5, |w|<=3.9, duplicated so sin/cos
stay lane-aligned, pi/2 phase pre-added via a ones-row matmul) need three.

e is computed TRANSPOSED (keys on partitions): eT[j,i] = Fk-chunks^T @ Gq,
so exp(eT) lands directly in the a^T layout the value matmul wants -- no
PE transposes of the attention matrix.  Softmax row-sums come from a ones
column appended to x (v_ps[:, D] accumulates sum_j a^T[j,i]).

Sharding: 8 cores = 4 batches x 2 query-halves, data-parallel.  Each core
gets x ROTATED so its own 512 queries are rows 0:511 (softmax over keys is
permutation-invariant), so one SPMD program serves all cores and the query
slice of xT is reused for both q and k paths.
"""

import numpy as np

import concourse.bass as bass
import concourse.mybir as mybir
import concourse.tile as tile
from concourse import bacc
from concourse.bass import ds, ts

B, L, D, U = 4, 1024, 256, 32
NCORES = 8
HALVES = 2
LQ = L // HALVES                # 512 queries per core
NJC = L // 128                  # 8 key chunks
NIB = LQ // 128                 # 4 query blocks
NDC = D // 128                  # 2 contraction chunks
NSL = 2                         # 512-wide key slices for the prologue
DP = D + 4                      # x padded: ones col at D, zeros after

# tanh(s) ~= ALIN*s + sum_m CFIT[m]*sin(OMEGA[m]*s) on s in [-8.8, 8.8].
# OMEGA[0:4] <= 1.837 (single wrap); OMEGA[4:6] <= 3.98 (three wraps).
OMEGA = np.array([0.7324021525072713, 0.9511720747858197, 1.04976141106319,
                  1.8371891778362637, 2.5118842414849865, 3.31911764103443])
CFIT = np.array([0.9486979585025787, -1.2708776193410671, 1.0006097137207512,
                 0.07638186974224523, 0.026376476065886594,
                 0.011125693292597548])
ALIN = 0.20894155850363957
M = len(OMEGA)                  # 6 frequencies
MU0 = 128                       # rows (m=0..3, u) -- block 0
MW = 256                        # weight cols: block0 + duplicated block1
NFC = 3                         # feature chunks of 128
FP16 = mybir.dt.float16
F32 = mybir.dt.float32
F32R = mybir.dt.float32r
AF = mybir.ActivationFunctionType
PI = float(np.pi)
HALF_PI = float(np.pi / 2.0)


def build_kernel(nc: bass.Bass, taps: bool = False):
    x_d = nc.dram_tensor("x", [L, DP], F32R, kind="ExternalInput")
    wxs_d = nc.dram_tensor("wxs", [D, MW], F32R, kind="ExternalInput")
    wts_d = nc.dram_tensor("wts", [D, MW], F32R, kind="ExternalInput")
    wlin_d = nc.dram_tensor("wlin", [D, 4], F32R, kind="ExternalInput")
    cw_d = nc.dram_tensor("cw", [128, 2], F32, kind="ExternalInput")
    phb_d = nc.dram_tensor("phb", [1, 3, 128], F32R, kind="ExternalInput")
    ones_d = nc.dram_tensor("ones", [1, LQ], F32R, kind="ExternalInput")
    ident_d = nc.dram_tensor("ident", [128, 128], F32R, kind="ExternalInput")
    out_d = nc.dram_tensor("out", [LQ, D], F32, kind="ExternalOutput")
    if taps:
        fk_t = nc.dram_tensor("fk_t", [128, NFC, L], F32, kind="ExternalOutput")
        gq_t = nc.dram_tensor("gq_t", [128, NFC, LQ], F32, kind="ExternalOutput")
        pt_t = nc.dram_tensor("pt_t", [128, NJC, LQ], F32R, kind="ExternalOutput")
        bs_t = nc.dram_tensor("bs_t", [128, 4 * NJC], F32, kind="ExternalOutput")

    with tile.TileContext(nc) as tc:
        with tc.tile_pool(name="const", bufs=1) as cpool:
            x_sb = cpool.tile([128, NJC, DP], F32R)
            xT_sb = cpool.tile([128, NDC, L], F32R)
            wxs_sb = cpool.tile([128, NDC, MW], F32R)
            wts_sb = cpool.tile([128, NDC, MW], F32R)
            wlin_sb = cpool.tile([128, NDC, 4], F32R)
            cw_sb = cpool.tile([128, 2], F32)
            phb_sb = cpool.tile([1, 3, 128], F32R)
            ones_sb = cpool.tile([1, LQ], F32R)
            ident_sb = cpool.tile([128, 128], F32R)
            fk_sb = cpool.tile([128, NFC, L], FP16)
            gq_sb = cpool.tile([128, NFC, LQ], FP16)
            pt_sb = cpool.tile([128, NJC, LQ], F32R)
            bsum_sb = cpool.tile([128, 4 * NJC], F32)
            recip_sb = cpool.tile([128, NIB], F32)

            # small/critical DMAs first
            nc.scalar.dma_start(ident_sb[:], ident_d.ap())
            nc.scalar.dma_start(cw_sb[:], cw_d.ap())
            nc.scalar.dma_start(phb_sb[:], phb_d.ap())
            nc.scalar.dma_start(ones_sb[:], ones_d.ap())
            nc.scalar.dma_start(
                wlin_sb[:], wlin_d.ap().rearrange("(c p) o -> p c o", p=128)
            )
            nc.sync.dma_start(
                wxs_sb[:], wxs_d.ap().rearrange("(c p) f -> p c f", p=128)
            )
            nc.sync.dma_start(
                wts_sb[:], wts_d.ap().rearrange("(c p) f -> p c f", p=128)
            )
            x_r = x_d.ap().rearrange("(c p) d -> c p d", p=128)
            for jc in (0, 1):
                nc.sync.dma_start(x_sb[:, jc, :], x_r[jc])
            for jc in (2, 3):
                nc.scalar.dma_start(x_sb[:, jc, :], x_r[jc])
            for jc in range(4, NJC):
                nc.gpsimd.dma_start(x_sb[:, jc, :], x_r[jc])

            # ---- prologue: xT, scaled projections, trig features ----
            with (
                tc.tile_pool(name="ptr", bufs=2, space="PSUM") as ptr,
                tc.tile_pool(name="pk0", bufs=2, space="PSUM") as pk0,
                tc.tile_pool(name="pk1", bufs=2, space="PSUM") as pk1,
                tc.tile_pool(name="stage", bufs=2) as stg,
            ):
                bs_ps = pk0.tile([128, 4 * NJC], F32, tag="bs", bufs=1)
                for n in range(NSL):
                    # transpose one 512-key slice of x into xT
                    for dc in range(NDC):
                        tr4 = ptr.tile([128, 512], F32R, tag="tr")
                        for q4 in range(4):
                            jc = 4 * n + q4
                            nc.tensor.transpose(
                                tr4[:, ts(q4, 128)],
                                x_sb[:, jc, ds(dc * 128, 128)],
                                ident_sb[:],
                            )
                        nc.vector.tensor_copy(
                            xT_sb[:, dc, ds(n * 512, 512)], tr4[:]
                        )
                    # exp-bias columns B_j = x_j . wlin for this slice
                    for sj in range(4):
                        jb = 4 * n + sj
                        for dc in range(NDC):
                            nc.tensor.matmul(
                                bs_ps[:, ds(4 * jb, 4)],
                                xT_sb[:, dc, ds(jb * 128, 128)],
                                wlin_sb[:, dc, :],
                                start=(dc == 0),
                                stop=(dc == NDC - 1),
                            )
                    # kT_all = Wxs^T @ xT for this slice, both row blocks;
                    # the K=1 ones-row matmul pre-adds phase offsets
                    kp0 = pk0.tile([128, 512], F32, tag="k0")
                    kp1 = pk1.tile([128, 512], F32, tag="k1")
                    sl = ds(n * 512, 512)
                    for dc in range(NDC):
                        nc.tensor.matmul(
                            kp0[:],
                            wxs_sb[:, dc, 0:MU0],
                            xT_sb[:, dc, sl],
                            start=(dc == 0),
                            stop=(dc == NDC - 1),
                        )
                    for dc in range(NDC):
                        nc.tensor.matmul(
                            kp1[:],
                            wxs_sb[:, dc, ds(MU0, 128)],
                            xT_sb[:, dc, sl],
                            start=(dc == 0),
                            stop=False,
                        )
                    nc.tensor.matmul(
                        kp1[:],
                        phb_sb[0:1, 0, :],
                        ones_sb[:],
                        start=False,
                        stop=True,
                    )
                    # range reduction + trig features
                    w0 = stg.tile([128, 512], F32, tag="w0")
                    w0c = stg.tile([128, 512], F32, tag="w0c")
                    nc.vector.add_range_wrap(w0[:], kp0[:], 0.0, PI, 2 * PI)
                    nc.vector.add_range_wrap(w0c[:], w0[:], HALF_PI, PI, 2 * PI)
                    w1 = stg.tile([128, 512], F32, tag="w1")
                    nc.vector.add_range_wrap(w1[:], kp1[:], 0.0, PI, 2 * PI)
                    nc.vector.add_range_wrap(w1[:], w1[:], 0.0, PI, 2 * PI)
                    nc.vector.add_range_wrap(w1[:], w1[:], 0.0, PI, 2 * PI)
                    nc.scalar.activation(fk_sb[:, 0, sl], w0c[:], AF.Sin)
                    nc.scalar.activation(fk_sb[:, 1, sl], w0[:], AF.Sin)
                    nc.scalar.activation(fk_sb[:, 2, sl], w1[:], AF.Sin)
                    if n == 0:
                        # queries are rows 0:512 (x pre-rotated per core);
                        # w_m*bh_u bias and block1 phase pre-added via the
                        # ones-row matmuls, coefficient c_m*Wa_u after
                        qp0 = pk0.tile([128, 512], F32, tag="k0")
                        qp1 = pk1.tile([128, 512], F32, tag="k1")
                        for dc in range(NDC):
                            nc.tensor.matmul(
                                qp0[:],
                                wts_sb[:, dc, 0:MU0],
                                xT_sb[:, dc, 0:512],
                                start=(dc == 0),
                                stop=False,
                            )
                        nc.tensor.matmul(
                            qp0[:],
                            phb_sb[0:1, 2, :],
                            ones_sb[:],
                            start=False,
                            stop=True,
                        )
                        for dc in range(NDC):
                            nc.tensor.matmul(
                                qp1[:],
                                wts_sb[:, dc, ds(MU0, 128)],
                                xT_sb[:, dc, 0:512],
                                start=(dc == 0),
                                stop=False,
                            )
                        nc.tensor.matmul(
                            qp1[:],
                            phb_sb[0:1, 1, :],
                            ones_sb[:],
                            start=False,
                            stop=True,
                        )
                        g0 = stg.tile([128, 512], F32, tag="w0")
                        g0c = stg.tile([128, 512], F32, tag="w0c")
                        nc.vector.add_range_wrap(g0[:], qp0[:], 0.0, PI, 2 * PI)
                        nc.vector.add_range_wrap(
                            g0c[:], g0[:], HALF_PI, PI, 2 * PI
                        )
                        g1 = stg.tile([128, 512], F32, tag="w1")
                        nc.vector.add_range_wrap(g1[:], qp1[:], 0.0, PI, 2 * PI)
                        nc.vector.add_range_wrap(g1[:], g1[:], 0.0, PI, 2 * PI)
                        nc.vector.add_range_wrap(g1[:], g1[:], 0.0, PI, 2 * PI)
                        nc.scalar.activation(gq_sb[:, 0, :], g0[:], AF.Sin)
                        nc.scalar.activation(gq_sb[:, 1, :], g0c[:], AF.Sin)
                        nc.scalar.activation(gq_sb[:, 2, :], g1[:], AF.Sin)
                        nc.vector.tensor_scalar_mul(
                            gq_sb[:, 0, :], gq_sb[:, 0, :], cw_sb[:, ds(0, 1)]
                        )
                        nc.vector.tensor_scalar_mul(
                            gq_sb[:, 1, :], gq_sb[:, 1, :], cw_sb[:, ds(0, 1)]
                        )
                        nc.vector.tensor_scalar_mul(
                            gq_sb[:, 2, :], gq_sb[:, 2, :], cw_sb[:, ds(1, 1)]
                        )
                nc.vector.tensor_copy(bsum_sb[:], bs_ps[:])

            # ---- main loop: scores, softmax, weighted sum ----
            with (
                tc.tile_pool(name="pe", bufs=3, space="PSUM") as pe_e,
                tc.tile_pool(name="pv", bufs=1, space="PSUM") as pe_v,
                tc.tile_pool(name="vout", bufs=2) as vpool,
            ):
                v_ps = [
                    pe_v.tile([128, DP], F32, name=f"v_ps{ib}")
                    for ib in range(NIB)
                ]
                for jb in range(NJC):
                    e_ps = pe_e.tile([128, LQ], F32)
                    for c in range(NFC):
                        nc.tensor.matmul(
                            e_ps[:],
                            fk_sb[:, c, ds(jb * 128, 128)],
                            gq_sb[:, c, :],
                            start=(c == 0),
                            stop=(c == NFC - 1),
                        )
                    # exp(e + B_j): linear-term k-part enters as the bias
                    nc.scalar.activation(
                        pt_sb[:, jb, :], e_ps[:], AF.Exp,
                        bias=bsum_sb[:, ds(4 * jb, 1)],
                    )
                    for ib in range(NIB):
                        nc.tensor.matmul(
                            v_ps[ib][:],
                            pt_sb[:, jb, ds(ib * 128, 128)],
                            x_sb[:, jb, :],
                            start=(jb == 0),
                            stop=(jb == NJC - 1),
                        )
                out_r = out_d.ap().rearrange("(ib p) d -> ib p d", p=128)
                for ib in range(NIB):
                    nc.vector.reciprocal(
                        recip_sb[:, ds(ib, 1)], v_ps[ib][:, ds(D, 1)]
                    )
                    v_sb = vpool.tile([128, D], F32)
                    nc.vector.tensor_scalar_mul(
                        v_sb[:], v_ps[ib][:, 0:D], recip_sb[:, ds(ib, 1)]
                    )
                    nc.sync.dma_start(out_r[ib], v_sb[:])
                if taps:
                    fk32_sb = cpool.tile([128, NFC, L], F32)
                    gq32_sb = cpool.tile([128, NFC, LQ], F32)
                    nc.vector.tensor_copy(fk32_sb[:], fk_sb[:])
                    nc.vector.tensor_copy(gq32_sb[:], gq_sb[:])
                    nc.sync.dma_start(fk_t.ap(), fk32_sb[:])
                    nc.sync.dma_start(gq_t.ap(), gq32_sb[:])
                    nc.sync.dma_start(pt_t.ap(), pt_sb[:])
                    nc.sync.dma_start(bs_t.ap(), bsum_sb[:])

    return nc


_NC_CACHE: dict = {}


def get_compiled_nc():
    if "nc" not in _NC_CACHE:
        nc = bacc.Bacc("TRN2", target_bir_lowering=False, debug=False)
        build_kernel(nc)
        nc.compile()
        _NC_CACHE["nc"] = nc
    return _NC_CACHE["nc"]


def make_in_maps(inputs_np, Wt, Wx, bh, Wa):
    # scaled projection weights: column (m,u) = w_m * W[:, u]; the m=4,5
    # block is duplicated (cols 128:192 == 192:256) so both trig phases of
    # feature chunk 2 are computed lane-aligned
    wxs = np.empty((D, MW), np.float32)
    wts = np.empty((D, MW), np.float32)
    for m in range(M):
        wxs[:, 32 * m : 32 * (m + 1)] = OMEGA[m] * Wx
        wts[:, 32 * m : 32 * (m + 1)] = OMEGA[m] * Wt
    wxs[:, 192:MW] = wxs[:, 128:192]
    wts[:, 192:MW] = wts[:, 128:192]
    wlin = np.zeros((D, 4), np.float32)
    wlin[:, 0:1] = (ALIN * (Wx @ Wa)).astype(np.float32)
    # coefficient rows c_m*Wa_u: col 0 = m0..3; col 1 = m4,5 twice
    cwA = (CFIT[:4, None] * Wa[None, :, 0]).reshape(128)
    cwB1 = (CFIT[4:6, None] * Wa[None, :, 0]).reshape(64)
    cw = np.stack(
        [cwA, np.concatenate([cwB1, cwB1])], axis=1
    ).astype(np.float32)
    # ones-row pre-add vectors: [0] = k block1 phase [0;pi/2];
    # [1] = q block1 w_m*bh_u + phase; [2] = q block0 w_m*bh_u
    # (k block0 needs none: its cos phase rides on the shifted wrap)
    bhB1 = (OMEGA[4:6, None] * bh[None, :]).reshape(64)
    phb = np.zeros((1, 3, 128), np.float32)
    phb[0, 0, :64] = HALF_PI
    phb[0, 1, :64] = bhB1
    phb[0, 1, 64:] = bhB1 + HALF_PI
    phb[0, 2, :] = (OMEGA[:4, None] * bh[None, :]).reshape(128)
    ones = np.ones((1, LQ), np.float32)
    ident = np.eye(128, dtype=np.float32)
    in_maps = []
    for c in range(NCORES):
        b, half = divmod(c, HALVES)
        xb = inputs_np[b]
        x_rot = xb if not half else np.concatenate(
            [xb[half * LQ :], xb[: half * LQ]], axis=0
        )
        # col D = ones (accumulates softmax row-sums in the value matmul),
        # cols D+1.. = zero padding for ISA-legal matmul widths
        pad = np.zeros((L, DP - D), np.float32)
        pad[:, 0] = 1.0
        x_rot = np.ascontiguousarray(np.concatenate([x_rot, pad], axis=1))
        in_maps.append(
            {
                "x": x_rot,
                "wxs": wxs,
                "wts": wts,
                "wlin": wlin,
                "cw": cw,
                "phb": phb,
                "ones": ones,
                "ident": ident,
            }
        )
    return in_maps


def kernel(**inputs) -> np.ndarray:
    x = np.asarray(inputs["inputs"], dtype=np.float32)
    Wt = np.ascontiguousarray(np.asarray(inputs["Wt"], np.float32))
    Wx = np.ascontiguousarray(np.asarray(inputs["Wx"], np.float32))
    bh = np.asarray(inputs["bh"], np.float32)
    Wa = np.asarray(inputs["Wa"], np.float32)

    from concourse.bass_utils import run_bass_kernel_spmd

    nc = get_compiled_nc()
    in_maps = make_in_maps(x, Wt, Wx, bh, Wa)
    res = run_bass_kernel_spmd(nc, in_maps, list(range(NCORES)))
    kernel._last_results = res  # type: ignore[attr-defined]

    out = np.empty((B, L, D), np.float32)
    for c in range(NCORES):
        b, half = divmod(c, HALVES)
        out[b, half * LQ : (half + 1) * LQ] = res.results[c]["out"]
    return out


# revision 19
# speedup vs baseline: 1.0645x; 1.0645x over previous
"""Bahdanau additive-attention pooling for Trainium2 (Bass/Tile).

Reference math (per batch):
    q = x @ Wt + bh; k = x @ Wx                             [L, U]
    e[i,j] = sum_u Wa[u] * tanh(q[i,u] + k[j,u])            (+ ba, dropped --
                                                             softmax shift-inv)
    v = softmax_j(e) @ x                                    [L, D]

Key trick: tanh is replaced by a fitted expansion
    tanh(s) ~= ALIN*s + sum_m CFIT[m] sin(OMEGA[m] s)
which SEPARATES over s = q + k:
    sin(w(q+k)) = sin(wq)cos(wk) + cos(wq)sin(wk)
so e becomes one matmul over F = 2*M*U = 384 trig features:
    e[i,j] ~= sum_f Gq[i,f]*Fk[j,f] + ALIN*(A_i + B_j)
    Gq[i,(m,u,ph)] = c_m*Wa_u*trig_ph(w_m q[i,u]),  Fk = cotrig_ph(w_m k[j,u])
The linear term's A_i part is constant per softmax row (drops out); B_j =
ALIN*(k@Wa)_j rides for free as a per-partition bias on the exp activation.
This removes the L*L*U elementwise tanh volume (the old ScalarE bottleneck,
~110us/core); scores cost one PE matmul with contraction dim 384.

The ScalarE Sin spline is only valid on [-pi, pi] (no hardware range
reduction), so projection tiles are folded with chained DVE add_range_wrap
ops: block0 rows (m=0..3, |w|<=1.83) need one wrap (plus one shifted wrap
for the cos phase); block1 rows (m=4,5, |w|<=3.9, duplicated so sin/cos
stay lane-aligned, pi/2 phase pre-added via a ones-row matmul) need three.

e is computed TRANSPOSED (keys on partitions): eT[j,i] = Fk-chunks^T @ Gq,
so exp(eT) lands directly in the a^T layout the value matmul wants -- no
PE transposes of the attention matrix.  Softmax row-sums come from a ones
column appended to x (v_ps[:, D] accumulates sum_j a^T[j,i]).

Sharding: 8 cores = 4 batches x 2 query-halves, data-parallel.  Each core
gets x ROTATED so its own 512 queries are rows 0:511 (softmax over keys is
permutation-invariant), so one SPMD program serves all cores and the query
slice of xT is reused for both q and k paths.
"""

import numpy as np

import concourse.bass as bass
import concourse.mybir as mybir
import concourse.tile as tile
from concourse import bacc
from concourse.bass import ds, ts

B, L, D, U = 4, 1024, 256, 32
NCORES = 8
HALVES = 2
LQ = L // HALVES                # 512 queries per core
NJC = L // 128                  # 8 key chunks
NIB = LQ // 128                 # 4 query blocks
NDC = D // 128                  # 2 contraction chunks
NSL = 2                         # 512-wide key slices for the prologue
DP = D + 4                      # x padded: ones col at D, zeros after

# tanh(s) ~= ALIN*s + sum_m CFIT[m]*sin(OMEGA[m]*s) on s in [-8.8, 8.8].
# OMEGA[0:4] <= 1.837 (single wrap); OMEGA[4:6] <= 3.98 (three wraps).
OMEGA = np.array([0.7324021525072713, 0.9511720747858197, 1.04976141106319,
                  1.8371891778362637, 2.5118842414849865, 3.31911764103443])
CFIT = np.array([0.9486979585025787, -1.2708776193410671, 1.0006097137207512,
                 0.07638186974224523, 0.026376476065886594,
                 0.011125693292597548])
ALIN = 0.20894155850363957
M = len(OMEGA)                  # 6 frequencies
MU0 = 128                       # rows (m=0..3, u) -- block 0
MW = 256                        # weight cols: block0 + duplicated block1
NFC = 3                         # feature chunks of 128
FP16 = mybir.dt.float16
F32 = mybir.dt.float32
F32R = mybir.dt.float32r
AF = mybir.ActivationFunctionType
PI = float(np.pi)
HALF_PI = float(np.pi / 2.0)


def build_kernel(nc: bass.Bass, taps: bool = False):
    x_d = nc.dram_tensor("x", [L, DP], F32R, kind="ExternalInput")
    wxs_d = nc.dram_tensor("wxs", [D, MW], F32R, kind="ExternalInput")
    wts_d = nc.dram_tensor("wts", [D, MW], F32R, kind="ExternalInput")
    wlin_d = nc.dram_tensor("wlin", [D, 4], F32R, kind="ExternalInput")
    cw_d = nc.dram_tensor("cw", [128, 2], F32, kind="ExternalInput")
    phb_d = nc.dram_tensor("phb", [1, 3, 128], F32R, kind="ExternalInput")
    ones_d = nc.dram_tensor("ones", [1, LQ], F32R, kind="ExternalInput")
    ident_d = nc.dram_tensor("ident", [128, 128], F32R, kind="ExternalInput")
    out_d = nc.dram_tensor("out", [LQ, D], F32, kind="ExternalOutput")
    if taps:
        fk_t = nc.dram_tensor("fk_t", [128, NFC, L], F32, kind="ExternalOutput")
        gq_t = nc.dram_tensor("gq_t", [128, NFC, LQ], F32, kind="ExternalOutput")
        pt_t = nc.dram_tensor("pt_t", [128, NJC, LQ], F32R, kind="ExternalOutput")
        bs_t = nc.dram_tensor("bs_t", [128, 4 * NJC], F32, kind="ExternalOutput")

    with tile.TileContext(nc) as tc:
        with tc.tile_pool(name="const", bufs=1) as cpool:
            x_sb = cpool.tile([128, NJC, DP], F32R)
            xT_sb = cpool.tile([128, NDC, L], F32R)
            wxs_sb = cpool.tile([128, NDC, MW], F32R)
            wts_sb = cpool.tile([128, NDC, MW], F32R)
            wlin_sb = cpool.tile([128, NDC, 4], F32R)
            cw_sb = cpool.tile([128, 2], F32)
            phb_sb = cpool.tile([1, 3, 128], F32R)
            ones_sb = cpool.tile([1, LQ], F32R)
            ident_sb = cpool.tile([128, 128], F32R)
            fk_sb = cpool.tile([128, NFC, L], FP16)
            gq_sb = cpool.tile([128, NFC, LQ], FP16)
            pt_sb = cpool.tile([128, NJC, LQ], F32R)
            bsum_sb = cpool.tile([128, 4 * NJC], F32)
            recip_sb = cpool.tile([128, NIB], F32)

            # x chunks and the transpose identity first -- they gate the
            # whole prologue; weights and small constants follow
            x_r = x_d.ap().rearrange("(c p) d -> c p d", p=128)
            nc.scalar.dma_start(ident_sb[:], ident_d.ap())
            for jc in (0, 2):
                nc.sync.dma_start(x_sb[:, jc, :], x_r[jc])
            for jc in (1, 3):
                nc.scalar.dma_start(x_sb[:, jc, :], x_r[jc])
            for jc in (4, 5):
                nc.gpsimd.dma_start(x_sb[:, jc, :], x_r[jc])
            for jc in (6, 7):
                nc.sync.dma_start(x_sb[:, jc, :], x_r[jc])
            nc.sync.dma_start(
                wxs_sb[:], wxs_d.ap().rearrange("(c p) f -> p c f", p=128)
            )
            nc.scalar.dma_start(
                wts_sb[:], wts_d.ap().rearrange("(c p) f -> p c f", p=128)
            )
            nc.gpsimd.dma_start(cw_sb[:], cw_d.ap())
            nc.gpsimd.dma_start(phb_sb[:], phb_d.ap())
            nc.gpsimd.dma_start(ones_sb[:], ones_d.ap())
            nc.gpsimd.dma_start(
                wlin_sb[:], wlin_d.ap().rearrange("(c p) o -> p c o", p=128)
            )

            # ---- prologue: xT, scaled projections, trig features ----
            with (
                tc.tile_pool(name="ptr", bufs=2, space="PSUM") as ptr,
                tc.tile_pool(name="pk0", bufs=2, space="PSUM") as pk0,
                tc.tile_pool(name="pk1", bufs=2, space="PSUM") as pk1,
                tc.tile_pool(name="stage", bufs=2) as stg,
            ):
                bs_ps = pk0.tile([128, 4 * NJC], F32, tag="bs", bufs=1)
                for n in range(NSL):
                    # transpose one 512-key slice of x into xT
                    for dc in range(NDC):
                        tr4 = ptr.tile([128, 512], F32R, tag="tr")
                        for q4 in range(4):
                            jc = 4 * n + q4
                            nc.tensor.transpose(
                                tr4[:, ts(q4, 128)],
                                x_sb[:, jc, ds(dc * 128, 128)],
                                ident_sb[:],
                            )
                        if dc == 0:
                            nc.vector.tensor_copy(
                                xT_sb[:, dc, ds(n * 512, 512)], tr4[:]
                            )
                        else:
                            nc.scalar.copy(
                                xT_sb[:, dc, ds(n * 512, 512)], tr4[:]
                            )
                    # exp-bias columns B_j = x_j . wlin for this slice
                    for sj in range(4):
                        jb = 4 * n + sj
                        for dc in range(NDC):
                            nc.tensor.matmul(
                                bs_ps[:, ds(4 * jb, 4)],
                                xT_sb[:, dc, ds(jb * 128, 128)],
                                wlin_sb[:, dc, :],
                                start=(dc == 0),
                                stop=(dc == NDC - 1),
                            )
                    # kT_all = Wxs^T @ xT for this slice, both row blocks;
                    # the K=1 ones-row matmul pre-adds phase offsets
                    kp0 = pk0.tile([128, 512], F32, tag="k0")
                    kp1 = pk1.tile([128, 512], F32, tag="k1")
                    sl = ds(n * 512, 512)
                    for dc in range(NDC):
                        nc.tensor.matmul(
                            kp0[:],
                            wxs_sb[:, dc, 0:MU0],
                            xT_sb[:, dc, sl],
                            start=(dc == 0),
                            stop=(dc == NDC - 1),
                        )
                    for dc in range(NDC):
                        nc.tensor.matmul(
                            kp1[:],
                            wxs_sb[:, dc, ds(MU0, 128)],
                            xT_sb[:, dc, sl],
                            start=(dc == 0),
                            stop=False,
                        )
                    nc.tensor.matmul(
                        kp1[:],
                        phb_sb[0:1, 0, :],
                        ones_sb[:],
                        start=False,
                        stop=True,
                    )
                    # range reduction + trig features
                    w0 = stg.tile([128, 512], F32, tag="w0")
                    w0c = stg.tile([128, 512], F32, tag="w0c")
                    nc.vector.add_range_wrap(w0[:], kp0[:], 0.0, PI, 2 * PI)
                    nc.vector.add_range_wrap(w0c[:], w0[:], HALF_PI, PI, 2 * PI)
                    w1 = stg.tile([128, 512], F32, tag="w1")
                    nc.vector.add_range_wrap(w1[:], kp1[:], 0.0, 3 * PI, 4 * PI)
                    nc.vector.add_range_wrap(w1[:], w1[:], 0.0, PI, 2 * PI)
                    nc.scalar.activation(fk_sb[:, 0, sl], w0c[:], AF.Sin)
                    nc.scalar.activation(fk_sb[:, 1, sl], w0[:], AF.Sin)
                    nc.scalar.activation(fk_sb[:, 2, sl], w1[:], AF.Sin)
                    if n == 0:
                        # queries are rows 0:512 (x pre-rotated per core);
                        # w_m*bh_u bias and block1 phase pre-added via the
                        # ones-row matmuls, coefficient c_m*Wa_u after
                        qp0 = pk0.tile([128, 512], F32, tag="k0")
                        qp1 = pk1.tile([128, 512], F32, tag="k1")
                        for dc in range(NDC):
                            nc.tensor.matmul(
                                qp0[:],
                                wts_sb[:, dc, 0:MU0],
                                xT_sb[:, dc, 0:512],
                                start=(dc == 0),
                                stop=False,
                            )
                        nc.tensor.matmul(
                            qp0[:],
                            phb_sb[0:1, 2, :],
                            ones_sb[:],
                            start=False,
                            stop=True,
                        )
                        for dc in range(NDC):
                            nc.tensor.matmul(
                                qp1[:],
                                wts_sb[:, dc, ds(MU0, 128)],
                                xT_sb[:, dc, 0:512],
                                start=(dc == 0),
                                stop=False,
                            )
                        nc.tensor.matmul(
                            qp1[:],
                            phb_sb[0:1, 1, :],
                            ones_sb[:],
                            start=False,
                            stop=True,
                        )
                        g0 = stg.tile([128, 512], F32, tag="w0")
                        g0c = stg.tile([128, 512], F32, tag="w0c")
                        nc.vector.add_range_wrap(g0[:], qp0[:], 0.0, PI, 2 * PI)
                        nc.vector.add_range_wrap(
                            g0c[:], g0[:], HALF_PI, PI, 2 * PI
                        )
                        g1 = stg.tile([128, 512], F32, tag="w1")
                        nc.vector.add_range_wrap(g1[:], qp1[:], 0.0, 3 * PI, 4 * PI)
                        nc.vector.add_range_wrap(g1[:], g1[:], 0.0, PI, 2 * PI)
                        nc.scalar.activation(gq_sb[:, 0, :], g0[:], AF.Sin)
                        nc.scalar.activation(gq_sb[:, 1, :], g0c[:], AF.Sin)
                        nc.scalar.activation(gq_sb[:, 2, :], g1[:], AF.Sin)
                        nc.vector.tensor_scalar_mul(
                            gq_sb[:, 0, :], gq_sb[:, 0, :], cw_sb[:, ds(0, 1)]
                        )
                        nc.vector.tensor_scalar_mul(
                            gq_sb[:, 1, :], gq_sb[:, 1, :], cw_sb[:, ds(0, 1)]
                        )
                        nc.vector.tensor_scalar_mul(
                            gq_sb[:, 2, :], gq_sb[:, 2, :], cw_sb[:, ds(1, 1)]
                        )
                nc.vector.tensor_copy(bsum_sb[:], bs_ps[:])

            # ---- main loop: scores, softmax, weighted sum ----
            with (
                tc.tile_pool(name="pe", bufs=3, space="PSUM") as pe_e,
                tc.tile_pool(name="pv", bufs=1, space="PSUM") as pe_v,
                tc.tile_pool(name="vout", bufs=2) as vpool,
            ):
                v_ps = [
                    pe_v.tile([128, DP], F32, name=f"v_ps{ib}")
                    for ib in range(NIB)
                ]
                jb_order = [4, 5, 6, 7, 0, 1, 2, 3]
                for ji, jb in enumerate(jb_order):
                    e_ps = pe_e.tile([128, LQ], F32)
                    for c in range(NFC):
                        nc.tensor.matmul(
                            e_ps[:],
                            fk_sb[:, c, ds(jb * 128, 128)],
                            gq_sb[:, c, :],
                            start=(c == 0),
                            stop=(c == NFC - 1),
                        )
                    # exp(e + B_j): linear-term k-part enters as the bias
                    nc.scalar.activation(
                        pt_sb[:, jb, :], e_ps[:], AF.Exp,
                        bias=bsum_sb[:, ds(4 * jb, 1)],
                    )
                    for ib in range(NIB):
                        nc.tensor.matmul(
                            v_ps[ib][:],
                            pt_sb[:, jb, ds(ib * 128, 128)],
                            x_sb[:, jb, :],
                            start=(ji == 0),
                            stop=(ji == NJC - 1),
                        )
                out_r = out_d.ap().rearrange("(ib p) d -> ib p d", p=128)
                for ib in range(NIB):
                    nc.vector.reciprocal(
                        recip_sb[:, ds(ib, 1)], v_ps[ib][:, ds(D, 1)]
                    )
                    v_sb = vpool.tile([128, D], F32)
                    nc.scalar.activation(
                        v_sb[:], v_ps[ib][:, 0:D], AF.Copy,
                        scale=recip_sb[:, ds(ib, 1)],
                    )
                    nc.sync.dma_start(out_r[ib], v_sb[:])
                if taps:
                    fk32_sb = cpool.tile([128, NFC, L], F32)
                    gq32_sb = cpool.tile([128, NFC, LQ], F32)
                    nc.vector.tensor_copy(fk32_sb[:], fk_sb[:])
                    nc.vector.tensor_copy(gq32_sb[:], gq_sb[:])
                    nc.sync.dma_start(fk_t.ap(), fk32_sb[:])
                    nc.sync.dma_start(gq_t.ap(), gq32_sb[:])
                    nc.sync.dma_start(pt_t.ap(), pt_sb[:])
                    nc.sync.dma_start(bs_t.ap(), bsum_sb[:])

    return nc


_NC_CACHE: dict = {}


def get_compiled_nc():
    if "nc" not in _NC_CACHE:
        nc = bacc.Bacc("TRN2", target_bir_lowering=False, debug=False)
        build_kernel(nc)
        nc.compile()
        _NC_CACHE["nc"] = nc
    return _NC_CACHE["nc"]


def make_in_maps(inputs_np, Wt, Wx, bh, Wa):
    # scaled projection weights: column (m,u) = w_m * W[:, u]; the m=4,5
    # block is duplicated (cols 128:192 == 192:256) so both trig phases of
    # feature chunk 2 are computed lane-aligned
    wxs = np.empty((D, MW), np.float32)
    wts = np.empty((D, MW), np.float32)
    for m in range(M):
        wxs[:, 32 * m : 32 * (m + 1)] = OMEGA[m] * Wx
        wts[:, 32 * m : 32 * (m + 1)] = OMEGA[m] * Wt
    wxs[:, 192:MW] = wxs[:, 128:192]
    wts[:, 192:MW] = wts[:, 128:192]
    wlin = np.zeros((D, 4), np.float32)
    wlin[:, 0:1] = (ALIN * (Wx @ Wa)).astype(np.float32)
    # coefficient rows c_m*Wa_u: col 0 = m0..3; col 1 = m4,5 twice
    cwA = (CFIT[:4, None] * Wa[None, :, 0]).reshape(128)
    cwB1 = (CFIT[4:6, None] * Wa[None, :, 0]).reshape(64)
    cw = np.stack(
        [cwA, np.concatenate([cwB1, cwB1])], axis=1
    ).astype(np.float32)
    # ones-row pre-add vectors: [0] = k block1 phase [0;pi/2];
    # [1] = q block1 w_m*bh_u + phase; [2] = q block0 w_m*bh_u
    # (k block0 needs none: its cos phase rides on the shifted wrap)
    bhB1 = (OMEGA[4:6, None] * bh[None, :]).reshape(64)
    phb = np.zeros((1, 3, 128), np.float32)
    phb[0, 0, :64] = HALF_PI
    phb[0, 1, :64] = bhB1
    phb[0, 1, 64:] = bhB1 + HALF_PI
    phb[0, 2, :] = (OMEGA[:4, None] * bh[None, :]).reshape(128)
    ones = np.ones((1, LQ), np.float32)
    ident = np.eye(128, dtype=np.float32)
    in_maps = []
    for c in range(NCORES):
        b, half = divmod(c, HALVES)
        xb = inputs_np[b]
        x_rot = xb if not half else np.concatenate(
            [xb[half * LQ :], xb[: half * LQ]], axis=0
        )
        # col D = ones (accumulates softmax row-sums in the value matmul),
        # cols D+1.. = zero padding for ISA-legal matmul widths
        pad = np.zeros((L, DP - D), np.float32)
        pad[:, 0] = 1.0
        x_rot = np.ascontiguousarray(np.concatenate([x_rot, pad], axis=1))
        in_maps.append(
            {
                "x": x_rot,
                "wxs": wxs,
                "wts": wts,
                "wlin": wlin,
                "cw": cw,
                "phb": phb,
                "ones": ones,
                "ident": ident,
            }
        )
    return in_maps


def kernel(**inputs) -> np.ndarray:
    x = np.asarray(inputs["inputs"], dtype=np.float32)
    Wt = np.ascontiguousarray(np.asarray(inputs["Wt"], np.float32))
    Wx = np.ascontiguousarray(np.asarray(inputs["Wx"], np.float32))
    bh = np.asarray(inputs["bh"], np.float32)
    Wa = np.asarray(inputs["Wa"], np.float32)

    from concourse.bass_utils import run_bass_kernel_spmd

    nc = get_compiled_nc()
    in_maps = make_in_maps(x, Wt, Wx, bh, Wa)
    res = run_bass_kernel_spmd(nc, in_maps, list(range(NCORES)))
    kernel._last_results = res  # type: ignore[attr-defined]

    out = np.empty((B, L, D), np.float32)
    for c in range(NCORES):
        b, half = divmod(c, HALVES)
        out[b, half * LQ : (half + 1) * LQ] = res.results[c]["out"]
    return out


# revision 20
# speedup vs baseline: 1.1189x; 1.0511x over previous
"""Bahdanau additive-attention pooling for Trainium2 (Bass/Tile).

Reference math (per batch):
    q = x @ Wt + bh; k = x @ Wx                             [L, U]
    e[i,j] = sum_u Wa[u] * tanh(q[i,u] + k[j,u])            (+ ba, dropped --
                                                             softmax shift-inv)
    v = softmax_j(e) @ x                                    [L, D]

Key trick: tanh is replaced by a fitted expansion
    tanh(s) ~= ALIN*s + sum_m CFIT[m] sin(OMEGA[m] s)
which SEPARATES over s = q + k:
    sin(w(q+k)) = sin(wq)cos(wk) + cos(wq)sin(wk)
so e becomes one matmul over F = 2*M*U = 384 trig features:
    e[i,j] ~= sum_f Gq[i,f]*Fk[j,f] + ALIN*(A_i + B_j)
    Gq[i,(m,u,ph)] = c_m*Wa_u*trig_ph(w_m q[i,u]),  Fk = cotrig_ph(w_m k[j,u])
The linear term's A_i part is constant per softmax row (drops out); B_j =
ALIN*(k@Wa)_j rides for free as a per-partition bias on the exp activation.
This removes the L*L*U elementwise tanh volume (the old ScalarE bottleneck,
~110us/core); scores cost one PE matmul with contraction dim 384.

The ScalarE Sin spline is only valid on [-pi, pi] (no hardware range
reduction), so projection tiles are folded with chained DVE add_range_wrap
ops: block0 rows (m=0..3, |w|<=1.83) need one wrap (plus one shifted wrap
for the cos phase); block1 rows (m=4,5, |w|<=3.9, duplicated so sin/cos
stay lane-aligned, pi/2 phase pre-added via a ones-row matmul) need three.

e is computed TRANSPOSED (keys on partitions): eT[j,i] = Fk-chunks^T @ Gq,
so exp(eT) lands directly in the a^T layout the value matmul wants -- no
PE transposes of the attention matrix.  Softmax row-sums come from a ones
column appended to x (v_ps[:, D] accumulates sum_j a^T[j,i]).

Sharding: 8 cores = 4 batches x 2 query-halves, data-parallel.  Each core
gets x ROTATED so its own 512 queries are rows 0:511 (softmax over keys is
permutation-invariant), so one SPMD program serves all cores and the query
slice of xT is reused for both q and k paths.
"""

import numpy as np

import concourse.bass as bass
import concourse.mybir as mybir
import concourse.tile as tile
from concourse import bacc
from concourse.bass import ds, ts

B, L, D, U = 4, 1024, 256, 32
NCORES = 8
HALVES = 2
LQ = L // HALVES                # 512 queries per core
NJC = L // 128                  # 8 key chunks
NIB = LQ // 128                 # 4 query blocks
NDC = D // 128                  # 2 contraction chunks
NSL = 2                         # 512-wide key slices for the prologue
DP = D + 4                      # x padded: ones col at D, zeros after

# tanh(s) ~= ALIN*s + sum_m CFIT[m]*sin(OMEGA[m]*s) on s in [-8.8, 8.8].
# OMEGA[0:4] <= 1.837 (single wrap); OMEGA[4:6] <= 3.98 (three wraps).
OMEGA = np.array([0.7324021525072713, 0.9511720747858197, 1.04976141106319,
                  1.8371891778362637, 2.5118842414849865, 3.31911764103443])
CFIT = np.array([0.9486979585025787, -1.2708776193410671, 1.0006097137207512,
                 0.07638186974224523, 0.026376476065886594,
                 0.011125693292597548])
ALIN = 0.20894155850363957
M = len(OMEGA)                  # 6 frequencies
MU0 = 128                       # rows (m=0..3, u) -- block 0
MW = 256                        # weight cols: block0 + duplicated block1
NFC = 3                         # feature chunks of 128
FP16 = mybir.dt.float16
F32 = mybir.dt.float32
F32R = mybir.dt.float32r
AF = mybir.ActivationFunctionType
PI = float(np.pi)
HALF_PI = float(np.pi / 2.0)


def build_kernel(nc: bass.Bass, taps: bool = False):
    x_d = nc.dram_tensor("x", [L, DP], F32R, kind="ExternalInput")
    wxs_d = nc.dram_tensor("wxs", [D, MW], F32R, kind="ExternalInput")
    wts_d = nc.dram_tensor("wts", [D, MW], F32R, kind="ExternalInput")
    wlin_d = nc.dram_tensor("wlin", [D, 4], F32R, kind="ExternalInput")
    cw_d = nc.dram_tensor("cw", [128, 2], F32, kind="ExternalInput")
    phb_d = nc.dram_tensor("phb", [1, 3, 128], F32R, kind="ExternalInput")
    ones_d = nc.dram_tensor("ones", [1, LQ], F32R, kind="ExternalInput")
    ident_d = nc.dram_tensor("ident", [128, 128], F32R, kind="ExternalInput")
    out_d = nc.dram_tensor("out", [LQ, D], F32, kind="ExternalOutput")
    if taps:
        fk_t = nc.dram_tensor("fk_t", [128, NFC, L], F32, kind="ExternalOutput")
        gq_t = nc.dram_tensor("gq_t", [128, NFC, LQ], F32, kind="ExternalOutput")
        pt_t = nc.dram_tensor("pt_t", [128, NJC, LQ], F32R, kind="ExternalOutput")
        bs_t = nc.dram_tensor("bs_t", [128, 4 * NJC], F32, kind="ExternalOutput")

    with tile.TileContext(nc) as tc:
        with tc.tile_pool(name="const", bufs=1) as cpool:
            x_sb = cpool.tile([128, NJC, DP], F32R)
            xT_sb = cpool.tile([128, NDC, L], F32R)
            wxs_sb = cpool.tile([128, NDC, MW], F32R)
            wts_sb = cpool.tile([128, NDC, MW], F32R)
            wlin_sb = cpool.tile([128, NDC, 4], F32R)
            cw_sb = cpool.tile([128, 2], F32)
            phb_sb = cpool.tile([1, 3, 128], F32R)
            ones_sb = cpool.tile([1, LQ], F32R)
            ident_sb = cpool.tile([128, 128], F32R)
            fk_sb = cpool.tile([128, NFC, L], FP16)
            gq_sb = cpool.tile([128, NFC, LQ], FP16)
            pt_sb = cpool.tile([128, NJC, LQ], F32R)
            bsum_sb = cpool.tile([128, 4 * NJC], F32)
            recip_sb = cpool.tile([128, NIB], F32)

            # x chunks and the transpose identity first -- they gate the
            # whole prologue; weights and small constants follow
            x_r2 = x_d.ap().rearrange("(g c p) d -> g p c d", c=2, p=128)
            nc.scalar.dma_start(ident_sb[:], ident_d.ap())
            nc.sync.dma_start(x_sb[:, 0:2, :], x_r2[0])
            nc.scalar.dma_start(x_sb[:, 2:4, :], x_r2[1])
            nc.sync.dma_start(
                wxs_sb[:], wxs_d.ap().rearrange("(c p) f -> p c f", p=128)
            )
            nc.scalar.dma_start(x_sb[:, 4:6, :], x_r2[2])
            nc.sync.dma_start(phb_sb[:], phb_d.ap())
            nc.sync.dma_start(ones_sb[:], ones_d.ap())
            nc.sync.dma_start(
                wlin_sb[:], wlin_d.ap().rearrange("(c p) o -> p c o", p=128)
            )
            nc.scalar.dma_start(x_sb[:, 6:8, :], x_r2[3])
            nc.sync.dma_start(
                wts_sb[:], wts_d.ap().rearrange("(c p) f -> p c f", p=128)
            )
            nc.sync.dma_start(cw_sb[:], cw_d.ap())

            # ---- prologue: xT, scaled projections, trig features ----
            with (
                tc.tile_pool(name="ptr", bufs=2, space="PSUM") as ptr,
                tc.tile_pool(name="pk0", bufs=2, space="PSUM") as pk0,
                tc.tile_pool(name="pk1", bufs=2, space="PSUM") as pk1,
                tc.tile_pool(name="stage", bufs=2) as stg,
            ):
                bs_ps = pk0.tile([128, 4 * NJC], F32, tag="bs", bufs=1)
                for n in range(NSL):
                    # transpose one 512-key slice of x into xT
                    for dc in range(NDC):
                        tr4 = ptr.tile([128, 512], F32R, tag="tr")
                        for q4 in range(4):
                            jc = 4 * n + q4
                            nc.tensor.transpose(
                                tr4[:, ts(q4, 128)],
                                x_sb[:, jc, ds(dc * 128, 128)],
                                ident_sb[:],
                            )
                        nc.vector.tensor_copy(
                            xT_sb[:, dc, ds(n * 512, 512)], tr4[:]
                        )
                    # exp-bias columns B_j = x_j . wlin for this slice
                    for sj in range(4):
                        jb = 4 * n + sj
                        for dc in range(NDC):
                            nc.tensor.matmul(
                                bs_ps[:, ds(4 * jb, 4)],
                                xT_sb[:, dc, ds(jb * 128, 128)],
                                wlin_sb[:, dc, :],
                                start=(dc == 0),
                                stop=(dc == NDC - 1),
                            )
                    # kT_all = Wxs^T @ xT for this slice, both row blocks;
                    # the K=1 ones-row matmul pre-adds phase offsets
                    kp0 = pk0.tile([128, 512], F32, tag="k0")
                    kp1 = pk1.tile([128, 512], F32, tag="k1")
                    sl = ds(n * 512, 512)
                    for dc in range(NDC):
                        nc.tensor.matmul(
                            kp0[:],
                            wxs_sb[:, dc, 0:MU0],
                            xT_sb[:, dc, sl],
                            start=(dc == 0),
                            stop=(dc == NDC - 1),
                        )
                    for dc in range(NDC):
                        nc.tensor.matmul(
                            kp1[:],
                            wxs_sb[:, dc, ds(MU0, 128)],
                            xT_sb[:, dc, sl],
                            start=(dc == 0),
                            stop=False,
                        )
                    nc.tensor.matmul(
                        kp1[:],
                        phb_sb[0:1, 0, :],
                        ones_sb[:],
                        start=False,
                        stop=True,
                    )
                    # range reduction + trig features
                    w0 = stg.tile([128, 512], F32, tag="w0")
                    w0c = stg.tile([128, 512], F32, tag="w0c")
                    nc.vector.add_range_wrap(w0[:], kp0[:], 0.0, PI, 2 * PI)
                    nc.vector.add_range_wrap(w0c[:], w0[:], HALF_PI, PI, 2 * PI)
                    w1 = stg.tile([128, 512], F32, tag="w1")
                    nc.vector.add_range_wrap(w1[:], kp1[:], 0.0, 3 * PI, 4 * PI)
                    nc.vector.add_range_wrap(w1[:], w1[:], 0.0, PI, 2 * PI)
                    nc.scalar.activation(fk_sb[:, 0, sl], w0c[:], AF.Sin)
                    nc.scalar.activation(fk_sb[:, 1, sl], w0[:], AF.Sin)
                    nc.scalar.activation(fk_sb[:, 2, sl], w1[:], AF.Sin)
                    if n == 0:
                        # queries are rows 0:512 (x pre-rotated per core);
                        # w_m*bh_u bias and block1 phase pre-added via the
                        # ones-row matmuls, coefficient c_m*Wa_u after
                        qp0 = pk0.tile([128, 512], F32, tag="k0")
                        qp1 = pk1.tile([128, 512], F32, tag="k1")
                        for dc in range(NDC):
                            nc.tensor.matmul(
                                qp0[:],
                                wts_sb[:, dc, 0:MU0],
                                xT_sb[:, dc, 0:512],
                                start=(dc == 0),
                                stop=False,
                            )
                        nc.tensor.matmul(
                            qp0[:],
                            phb_sb[0:1, 2, :],
                            ones_sb[:],
                            start=False,
                            stop=True,
                        )
                        for dc in range(NDC):
                            nc.tensor.matmul(
                                qp1[:],
                                wts_sb[:, dc, ds(MU0, 128)],
                                xT_sb[:, dc, 0:512],
                                start=(dc == 0),
                                stop=False,
                            )
                        nc.tensor.matmul(
                            qp1[:],
                            phb_sb[0:1, 1, :],
                            ones_sb[:],
                            start=False,
                            stop=True,
                        )
                        g0 = stg.tile([128, 512], F32, tag="w0")
                        g0c = stg.tile([128, 512], F32, tag="w0c")
                        nc.vector.add_range_wrap(g0[:], qp0[:], 0.0, PI, 2 * PI)
                        nc.vector.add_range_wrap(
                            g0c[:], g0[:], HALF_PI, PI, 2 * PI
                        )
                        g1 = stg.tile([128, 512], F32, tag="w1")
                        nc.vector.add_range_wrap(g1[:], qp1[:], 0.0, 3 * PI, 4 * PI)
                        nc.vector.add_range_wrap(g1[:], g1[:], 0.0, PI, 2 * PI)
                        nc.scalar.activation(gq_sb[:, 0, :], g0[:], AF.Sin)
                        nc.scalar.activation(gq_sb[:, 1, :], g0c[:], AF.Sin)
                        nc.scalar.activation(gq_sb[:, 2, :], g1[:], AF.Sin)
                        nc.vector.tensor_scalar_mul(
                            gq_sb[:, 0, :], gq_sb[:, 0, :], cw_sb[:, ds(0, 1)]
                        )
                        nc.vector.tensor_scalar_mul(
                            gq_sb[:, 1, :], gq_sb[:, 1, :], cw_sb[:, ds(0, 1)]
                        )
                        nc.vector.tensor_scalar_mul(
                            gq_sb[:, 2, :], gq_sb[:, 2, :], cw_sb[:, ds(1, 1)]
                        )
                nc.vector.scalar_tensor_tensor(
                    bsum_sb[:],
                    fk_sb[:, 2, ds(L - 4 * NJC, 4 * NJC)],
                    0.0,
                    bs_ps[:],
                    mybir.AluOpType.mult,
                    mybir.AluOpType.add,
                )

            # ---- main loop: scores, softmax, weighted sum ----
            with (
                tc.tile_pool(name="pe", bufs=3, space="PSUM") as pe_e,
                tc.tile_pool(name="pv", bufs=1, space="PSUM") as pe_v,
                tc.tile_pool(name="vout", bufs=2) as vpool,
            ):
                v_ps = [
                    pe_v.tile([128, DP], F32, name=f"v_ps{ib}")
                    for ib in range(NIB)
                ]
                for jb in range(NJC):
                    e_ps = pe_e.tile([128, LQ], F32)
                    for c in range(NFC):
                        nc.tensor.matmul(
                            e_ps[:],
                            fk_sb[:, c, ds(jb * 128, 128)],
                            gq_sb[:, c, :],
                            start=(c == 0),
                            stop=(c == NFC - 1),
                        )
                    # exp(e + B_j): linear-term k-part enters as the bias
                    nc.scalar.activation(
                        pt_sb[:, jb, :], e_ps[:], AF.Exp,
                        bias=bsum_sb[:, ds(4 * jb, 1)],
                    )
                    for ib in range(NIB):
                        nc.tensor.matmul(
                            v_ps[ib][:],
                            pt_sb[:, jb, ds(ib * 128, 128)],
                            x_sb[:, jb, :],
                            start=(jb == 0),
                            stop=(jb == NJC - 1),
                        )
                out_r = out_d.ap().rearrange("(ib p) d -> ib p d", p=128)
                for ib in range(NIB):
                    nc.vector.reciprocal(
                        recip_sb[:, ds(ib, 1)], v_ps[ib][:, ds(D, 1)]
                    )
                    v_sb = vpool.tile([128, D], F32)
                    nc.scalar.activation(
                        v_sb[:], v_ps[ib][:, 0:D], AF.Copy,
                        scale=recip_sb[:, ds(ib, 1)],
                    )
                    nc.sync.dma_start(out_r[ib], v_sb[:])
                if taps:
                    fk32_sb = cpool.tile([128, NFC, L], F32)
                    gq32_sb = cpool.tile([128, NFC, LQ], F32)
                    nc.vector.tensor_copy(fk32_sb[:], fk_sb[:])
                    nc.vector.tensor_copy(gq32_sb[:], gq_sb[:])
                    nc.sync.dma_start(fk_t.ap(), fk32_sb[:])
                    nc.sync.dma_start(gq_t.ap(), gq32_sb[:])
                    nc.sync.dma_start(pt_t.ap(), pt_sb[:])
                    nc.sync.dma_start(bs_t.ap(), bsum_sb[:])

    return nc


_NC_CACHE: dict = {}


def get_compiled_nc():
    if "nc" not in _NC_CACHE:
        nc = bacc.Bacc("TRN2", target_bir_lowering=False, debug=False)
        build_kernel(nc)
        nc.compile()
        _NC_CACHE["nc"] = nc
    return _NC_CACHE["nc"]


def make_in_maps(inputs_np, Wt, Wx, bh, Wa):
    # scaled projection weights: column (m,u) = w_m * W[:, u]; the m=4,5
    # block is duplicated (cols 128:192 == 192:256) so both trig phases of
    # feature chunk 2 are computed lane-aligned
    wxs = np.empty((D, MW), np.float32)
    wts = np.empty((D, MW), np.float32)
    for m in range(M):
        wxs[:, 32 * m : 32 * (m + 1)] = OMEGA[m] * Wx
        wts[:, 32 * m : 32 * (m + 1)] = OMEGA[m] * Wt
    wxs[:, 192:MW] = wxs[:, 128:192]
    wts[:, 192:MW] = wts[:, 128:192]
    wlin = np.zeros((D, 4), np.float32)
    wlin[:, 0:1] = (ALIN * (Wx @ Wa)).astype(np.float32)
    # coefficient rows c_m*Wa_u: col 0 = m0..3; col 1 = m4,5 twice
    cwA = (CFIT[:4, None] * Wa[None, :, 0]).reshape(128)
    cwB1 = (CFIT[4:6, None] * Wa[None, :, 0]).reshape(64)
    cw = np.stack(
        [cwA, np.concatenate([cwB1, cwB1])], axis=1
    ).astype(np.float32)
    # ones-row pre-add vectors: [0] = k block1 phase [0;pi/2];
    # [1] = q block1 w_m*bh_u + phase; [2] = q block0 w_m*bh_u
    # (k block0 needs none: its cos phase rides on the shifted wrap)
    bhB1 = (OMEGA[4:6, None] * bh[None, :]).reshape(64)
    phb = np.zeros((1, 3, 128), np.float32)
    phb[0, 0, :64] = HALF_PI
    phb[0, 1, :64] = bhB1
    phb[0, 1, 64:] = bhB1 + HALF_PI
    phb[0, 2, :] = (OMEGA[:4, None] * bh[None, :]).reshape(128)
    ones = np.ones((1, LQ), np.float32)
    ident = np.eye(128, dtype=np.float32)
    in_maps = []
    for c in range(NCORES):
        b, half = divmod(c, HALVES)
        xb = inputs_np[b]
        x_rot = xb if not half else np.concatenate(
            [xb[half * LQ :], xb[: half * LQ]], axis=0
        )
        # col D = ones (accumulates softmax row-sums in the value matmul),
        # cols D+1.. = zero padding for ISA-legal matmul widths
        pad = np.zeros((L, DP - D), np.float32)
        pad[:, 0] = 1.0
        x_rot = np.ascontiguousarray(np.concatenate([x_rot, pad], axis=1))
        in_maps.append(
            {
                "x": x_rot,
                "wxs": wxs,
                "wts": wts,
                "wlin": wlin,
                "cw": cw,
                "phb": phb,
                "ones": ones,
                "ident": ident,
            }
        )
    return in_maps


def kernel(**inputs) -> np.ndarray:
    x = np.asarray(inputs["inputs"], dtype=np.float32)
    Wt = np.ascontiguousarray(np.asarray(inputs["Wt"], np.float32))
    Wx = np.ascontiguousarray(np.asarray(inputs["Wx"], np.float32))
    bh = np.asarray(inputs["bh"], np.float32)
    Wa = np.asarray(inputs["Wa"], np.float32)

    from concourse.bass_utils import run_bass_kernel_spmd

    nc = get_compiled_nc()
    in_maps = make_in_maps(x, Wt, Wx, bh, Wa)
    res = run_bass_kernel_spmd(nc, in_maps, list(range(NCORES)))
    kernel._last_results = res  # type: ignore[attr-defined]

    out = np.empty((B, L, D), np.float32)
    for c in range(NCORES):
        b, half = divmod(c, HALVES)
        out[b, half * LQ : (half + 1) * LQ] = res.results[c]["out"]
    return out
